# revision 4
# baseline (speedup 1.0000x reference)
"""BiDAF block kernel for Trainium2 (Bass/Tile), data-parallel over batch on 8 cores.

Reference computation (see problem):
  attention-flow (trilinear) -> g [B,T,4H]
  3 stacked biLSTMs (l1: 4H->H, l2: 2H->H, lo: 2H->H)
  p1 = g@p1_wg + m@p1_wm ;  p2 = g@p2_wg + m2@p2_wm      (each [B,T])

Sharding: batch 32 split 4-per-core across 8 cores; weights replicated.
All heavy matmuls run as float32r (1 cyc/row at N>=256). Sequence-transposed
layouts are produced on the host (numpy) for inputs/weights, and on-chip via
PE transposes for activations.
"""

from contextlib import ExitStack

import numpy as np

import concourse.bacc as bacc
import concourse.bass as bass
import concourse.mybir as mybir
import concourse.tile as tile
from concourse.bass import ds, ts
from concourse.masks import make_identity

F32 = mybir.dt.float32
F32R = mybir.dt.float32r
BF16 = mybir.dt.bfloat16
AF = mybir.ActivationFunctionType
ALU = mybir.AluOpType
AX = mybir.AxisListType
P = 128

B_FULL, T_FULL, QLEN, H = 32, 384, 64, 768
H2, H4 = 2 * H, 4 * H
KH = H // P          # 6   (h-dim partition chunks)
KH2 = H2 // P        # 12
KH4 = H4 // P        # 24
N_CORES = 8

LAYERS = ("l1", "l2", "lo")
DIRS = ("f", "b")


def r(ap):
    """View an fp32 AP as float32r for full-rate PE streaming."""
    return ap.bitcast(F32R)


def build(ctx, tc, io, cfg):
    nc = tc.nc
    B = cfg["B"]          # local batch
    T = cfg["T"]
    SC = cfg["SC"]        # scan chunk (steps per hw-loop iteration)
    TC = T // P           # seq-dim 128-chunks
    NIT = T // SC
    Q = QLEN

    scal = cfg["scalars"]
    b_att = scal["b_att"]             # b_att_c + b_att_q + b_att_cq
    p_bias = {"p1": scal["p1_b"], "p2": scal["p2_b"]}

    # ---------------- DRAM scratch ----------------
    dram = ctx.enter_context(tc.tile_pool(name="dram", bufs=1, space="DRAM"))
    gT_d = dram.tile([B, KH4, P, T], BF16)              # g transposed (feat-part)
    xg_d = {d: dram.tile([B, T, H4], BF16, name=f"xg_{d}") for d in DIRS}  # input projections (per layer, reused)
    mT_d = {
        "l1": dram.tile([P, KH2, B, T], BF16, name="mT_l1"),
        "l2": dram.tile([P, KH2, B, T], BF16, name="mT_l2"),
        "lo": dram.tile([P, KH2, B, T], BF16, name="mT_lo"),
    }

    # ---------------- constants ----------------
    cpool = ctx.enter_context(tc.tile_pool(name="const", bufs=1))
    ident = cpool.tile([P, P], F32)
    make_identity(nc, ident)
    w_cq_sb = cpool.tile([P, KH], F32)
    nc.sync.dma_start(w_cq_sb, io["w_cq_"])
    w_c_sb = cpool.tile([P, KH], F32)
    nc.sync.dma_start(w_c_sb, io["w_c_"])
    w_q_sb = cpool.tile([P, KH], F32)
    nc.sync.dma_start(w_q_sb, io["w_q_"])
    ones_sb = cpool.tile([P, 1], F32)
    nc.vector.memset(ones_sb, 1.0)
    ones_row = cpool.tile([1, P], F32)
    nc.vector.memset(ones_row, 1.0)
    pw_sb = {}
    for nm, kc in (("p1_wg", KH4), ("p1_wm", KH2), ("p2_wg", KH4), ("p2_wm", KH2)):
        pw_sb[nm] = cpool.tile([P, kc], BF16, name=f"pw_{nm}")
        nc.sync.dma_start(pw_sb[nm], io[nm + "_"])

    # ================ Phase 1: attention -> gT ================
    with tc.tile_pool(name="att", bufs=2) as att, \
         tc.tile_pool(name="att_ps", bufs=4, space="PSUM") as aps:
        for b in range(B):
            cT_sb = att.tile([P, KH, T], F32, tag="cT")
            nc.sync.dma_start(cT_sb, io["cT"][b].rearrange("kc p t -> p kc t"))
            cna_sb = att.tile([P, TC, H], F32, tag="cna")
            nc.sync.dma_start(cna_sb, io["c"][b].rearrange("(io p) h -> p io h", p=P))
            q_sb = att.tile([Q, H], F32, tag="q")
            nc.sync.dma_start(q_sb, io["q"][b])
            qT_sb = att.tile([P, KH, Q], F32, tag="qT")
            nc.sync.dma_start(qT_sb, io["qT"][b].rearrange("kc p t -> p kc t"))

            # cw = cT * w_cq (broadcast over seq)
            cw_sb = att.tile([P, KH, T], F32, tag="cw")
            for k in range(KH):
                nc.vector.tensor_tensor(
                    cw_sb[:, k], cT_sb[:, k],
                    w_cq_sb[:, k, None].to_broadcast((P, T)), ALU.mult)

            # sq[j] = q @ w_att_q  -> [Q,1] -> row [1,Q]
            sq_ps = aps.tile([Q, 1], F32, tag="aps")
            for k in range(KH):
                nc.tensor.matmul(sq_ps, lhsT=qT_sb[:, k], rhs=w_q_sb[:, k, None],
                                 start=(k == 0), stop=(k == KH - 1))
            sq_col = att.tile([Q, 1], F32, tag="sq_col")
            nc.scalar.activation(sq_col, sq_ps, AF.Copy, bias=float(b_att))
            sqT_ps = aps.tile([1, Q], F32, tag="aps")
            nc.tensor.transpose(sqT_ps, sq_col, ident[:Q, :Q])
            sq_row = att.tile([1, Q], F32, tag="sq_row")
            nc.scalar.activation(sq_row, sqT_ps, AF.Copy)

            # per seq-chunk: s, softmax over q -> a ; row-max -> e2
            a_sb = att.tile([P, TC, Q], F32, tag="a")
            e2_sb = att.tile([P, TC], F32, tag="e2")
            for ic in range(TC):
                s_ps = aps.tile([P, Q], F32, tag="aps")
                for k in range(KH):
                    nc.tensor.matmul(s_ps, lhsT=cw_sb[:, k, ts(ic, P)],
                                     rhs=qT_sb[:, k],
                                     start=(k == 0), stop=False)
                nc.tensor.matmul(s_ps, lhsT=ones_row, rhs=sq_row,
                                 start=False, stop=True)
                sc_ps = aps.tile([P, 1], F32, tag="aps")
                for k in range(KH):
                    nc.tensor.matmul(sc_ps, lhsT=cT_sb[:, k, ts(ic, P)],
                                     rhs=w_c_sb[:, k, None],
                                     start=(k == 0), stop=(k == KH - 1))
                sc_sb = att.tile([P, 1], F32, tag="sc_sb")
                nc.scalar.activation(sc_sb, sc_ps, AF.Copy)
                s_sb = att.tile([P, Q], F32, tag="s_sb")
                nc.vector.tensor_tensor(s_sb, s_ps, sc_sb.to_broadcast((P, Q)), ALU.add)

                # softmax over free dim (q)
                nmx = att.tile([P, 1], F32, tag="nmx")
                nc.vector.reduce_max(nmx, s_sb, axis=AX.X, negate=True)
                nc.scalar.activation(a_sb[:, ic], s_sb, AF.Exp, bias=nmx)
                ssum = att.tile([P, 1], F32, tag="ssum")
                nc.vector.reduce_sum(ssum, a_sb[:, ic], axis=AX.X)
                rs = att.tile([P, 1], F32, tag="rs")
                nc.vector.reciprocal(rs, ssum)
                nc.vector.tensor_scalar_mul(a_sb[:, ic], a_sb[:, ic], rs)

                # row max of s (for b_w softmax over seq); no max-sub needed (|s| small)
                mx = att.tile([P, 1], F32, tag="mx")
                nc.vector.reduce_max(mx, s_sb, axis=AX.X)
                nc.scalar.activation(e2_sb[:, ic, None], mx, AF.Exp)

            # b_w = softmax over seq (partition-dim): sum via ones-matmul
            bsum_ps = aps.tile([1, TC], F32, tag="aps")
            nc.tensor.matmul(bsum_ps, lhsT=ones_sb, rhs=e2_sb, start=True, stop=True)
            tot = att.tile([1, 1], F32, tag="tot")
            nc.vector.reduce_sum(tot, bsum_ps, axis=AX.X)
            totb_ps = aps.tile([P, 1], F32, tag="aps")
            nc.tensor.matmul(totb_ps, lhsT=ones_row, rhs=tot, start=True, stop=True)
            rtot = att.tile([P, 1], F32, tag="rtot")
            nc.vector.reciprocal(rtot, totb_ps)
            bw_sb = att.tile([P, TC], F32, tag="bw")
            nc.vector.tensor_scalar_mul(bw_sb, e2_sb, rtot)

            # q2c = b_w @ c  -> [1, H]
            q2c_sb = att.tile([1, H], F32, tag="q2c_sb")
            for half in range(2):
                q2c_ps = aps.tile([1, H // 2], F32, tag="aps")
                for ic in range(TC):
                    nc.tensor.matmul(q2c_ps, lhsT=bw_sb[:, ic, None],
                                     rhs=cna_sb[:, ic, ds(half * (H // 2), H // 2)],
                                     start=(ic == 0), stop=(ic == TC - 1))
                nc.scalar.activation(q2c_sb[:, ds(half * (H // 2), H // 2)], q2c_ps, AF.Copy)
            # q2cT [P, KH]
            q2cT_sb = att.tile([P, KH], F32, tag="q2cT")
            for k in range(KH):
                q2cT_ps = aps.tile([P, 1], F32, tag="aps")
                nc.tensor.transpose(q2cT_ps, q2c_sb[:, ts(k, P)], ident[:1, :1])
                nc.scalar.activation(q2cT_sb[:, k, None], q2cT_ps, AF.Copy)

            # aT [Q, TC*P]
            aT_sb = att.tile([Q, TC, P], F32, tag="aT")
            for ic in range(TC):
                aT_ps = aps.tile([Q, P], F32, tag="aps")
                nc.tensor.transpose(aT_ps, a_sb[:, ic], ident)
                nc.scalar.activation(aT_sb[:, ic], aT_ps, AF.Copy)

            # c2qT per feature chunk + assemble g chunks, store to DRAM
            aT_flat = aT_sb.rearrange("q a b -> q (a b)")
            for fc in range(KH):
                c2q_ps = aps.tile([P, T], F32, tag="aps")
                nc.tensor.matmul(c2q_ps, lhsT=q_sb[:, ts(fc, P)], rhs=aT_flat,
                                 start=True, stop=True)
                c2q_sb = att.tile([P, T], F32, tag="c2q_sb")
                nc.scalar.activation(c2q_sb, c2q_ps, AF.Copy)
                c2qb_sb = att.tile([P, T], BF16, tag="c2qb_sb")
                nc.scalar.activation(c2qb_sb, c2q_ps, AF.Copy)
                g3_sb = att.tile([P, T], BF16, tag="g3")
                nc.vector.tensor_tensor(g3_sb, cT_sb[:, fc], c2q_sb, ALU.mult)
                g4_sb = att.tile([P, T], BF16, tag="g4")
                nc.vector.tensor_tensor(
                    g4_sb, cT_sb[:, fc],
                    q2cT_sb[:, fc, None].to_broadcast((P, T)), ALU.mult)
                nc.sync.dma_start(gT_d[b, fc], io["cT_bf"][b, fc])
                nc.sync.dma_start(gT_d[b, KH + fc], c2qb_sb)
                nc.sync.dma_start(gT_d[b, 2 * KH + fc], g3_sb)
                nc.sync.dma_start(gT_d[b, 3 * KH + fc], g4_sb)

    # ================ Phase 2: layers ================
    for li, lname in enumerate(LAYERS):
        srcT = gT_d if li == 0 else mT_d[LAYERS[li - 1]]
        KC = KH4 if li == 0 else KH2
        halves = 2 if KC == KH4 else 1
        HN = H4 // halves
        NB = HN // 512

        # ---- 2a: xg = src @ wihT + bias  (per dir) -> xg_d ----
        with tc.tile_pool(name=f"prj{li}", bufs=2) as prj, \
             tc.tile_pool(name=f"prjw{li}", bufs=1) as prjw, \
             tc.tile_pool(name=f"prj{li}_ps", bufs=2, space="PSUM") as pps:
            for d in DIRS:
                bias_bc = None
                if not cfg["bias_zero"][f"{lname}{d}"]:
                    bias_sb = prj.tile([1, H4], F32, tag="bias", name="bias_sb")
                    nc.sync.dma_start(bias_sb, io[f"{lname}{d}_bias"])
                    bias_bc = prj.tile([P, H4], F32, tag="bias_bc", name="bias_bc")
                    for n in range(H4 // 512):
                        bb_ps = pps.tile([P, 512], F32, tag="xg", name="bb_ps")
                        nc.tensor.matmul(bb_ps, lhsT=ones_row,
                                         rhs=bias_sb[:, ts(n, 512)],
                                         start=True, stop=True)
                        nc.scalar.activation(bias_bc[:, ts(n, 512)], bb_ps, AF.Copy)
                for half in range(halves):
                    w_sb = prjw.tile([P, KC, HN], BF16, tag="wih")
                    nc.sync.dma_start(
                        w_sb,
                        io[f"{lname}{d}_wihT"][:, :, ds(half * HN, HN)]
                        .rearrange("kc p n -> p kc n"))
                    for b in range(B):
                        for mc in range(TC):
                            inp_sb = prj.tile([P, KC, P], BF16, tag="inp")
                            if li == 0:
                                src_ap = srcT[b, :, :, ts(mc, P)].rearrange(
                                    "kc p t -> p kc t")
                            else:
                                src_ap = srcT[:, :, b, ts(mc, P)]
                            nc.sync.dma_start(inp_sb, src_ap)
                            for n in range(NB):
                                xg_ps = pps.tile([P, 512], F32, tag="xg")
                                for k in range(KC):
                                    nc.tensor.matmul(
                                        xg_ps, lhsT=inp_sb[:, k],
                                        rhs=w_sb[:, k, ts(n, 512)],
                                        start=(k == 0), stop=(k == KC - 1))
                                xg_sb = prj.tile([P, 512], BF16, tag="xg_sb")
                                off = half * HN + n * 512
                                if bias_bc is None:
                                    nc.scalar.activation(xg_sb, xg_ps, AF.Copy)
                                else:
                                    nc.vector.tensor_tensor(
                                        xg_sb, xg_ps,
                                        bias_bc[:, ds(off, 512)], ALU.add)
                                nc.sync.dma_start(
                                    xg_d[d][b, ts(mc, P), ds(off, 512)], xg_sb)

        if cfg.get("debug") and li == 0:
            nc.sync.dma_start(io["dbg_xg1f"], xg_d["f"][:])

        # ---- 2b: bidirectional scan (col-tiled, slab-packed) ----
        # Per dir: psum X [128,384] col-groups = slabs (i0,i1,f0,f1),
        #          psum Y [128,384] col-groups = slabs (g0,g1,o0,o1).
        # Group g occupies rows [32g, 32g+32) (M=32 stationary, cols 4..31 zero).
        NS = 384                       # slab width
        with tc.tile_pool(name=f"whh{li}", bufs=1) as whhp, \
             tc.tile_pool(name=f"st{li}", bufs=1) as stp, \
             tc.tile_pool(name=f"scan{li}", bufs=2) as scp, \
             tc.tile_pool(name=f"xgs{li}", bufs=2) as xgsp, \
             tc.tile_pool(name=f"scan{li}_ps", bufs=2, space="PSUM") as sps:
            whh_sb = {}
            c_pk = {}
            for d in DIRS:
                whh_sb[d] = whhp.tile([P, KH, H4], BF16, tag=f"whh_{d}", name=f"whh_{d}")
                nc.sync.dma_start(
                    whh_sb[d],
                    io[f"{lname}{d}_whhT"].rearrange("kc p n -> p kc n"))
                # packed cell state: rows 32u+b = c[b, 384u:384u+384]
                c_pk[d] = stp.tile([64, NS], F32, tag=f"c_{d}", name=f"c_{d}")
                nc.vector.memset(c_pk[d], 0.0)
            # padded stationary h^T: [128, dir, k, 32] (cols 4..31 stay zero)
            hT_pad = stp.tile([P, 2, KH, 32], BF16, tag="hT_pad", name="hT_pad")
            nc.vector.memset(hT_pad, 0.0)

            dma_engs = (nc.sync, nc.gpsimd, nc.scalar)
            with tc.For_i(0, NIT, 1) as iv:
                acc = scp.tile([P, 2, KH, B, SC], BF16, tag="acc", name="acc")
                # stage xg for this block: rows 32g+b <- xg[d][b, t, 384s]
                xgX, xgY = {}, {}
                ne = 0
                for di, d in enumerate(DIRS):
                    xgX[d] = xgsp.tile([P, SC, NS], BF16, tag=f"xgX_{d}", name=f"xgX_{d}")
                    xgY[d] = xgsp.tile([P, SC, NS], BF16, tag=f"xgY_{d}", name=f"xgY_{d}")
                    for g in range(4):
                        if d == "f":
                            srcX = xg_d[d][:, ds(iv * SC, SC), ds(NS * g, NS)]
                            srcY = xg_d[d][:, ds(iv * SC, SC), ds(NS * (4 + g), NS)]
                        else:
                            t0b = (T - SC) - iv * SC
                            srcX = xg_d[d][:, ds(t0b, SC), ds(NS * g, NS)][:, ::-1, :]
                            srcY = xg_d[d][:, ds(t0b, SC), ds(NS * (4 + g), NS)][:, ::-1, :]
                        dma_engs[ne % 3].dma_start(xgX[d][ds(32 * g, B)], srcX); ne += 1
                        dma_engs[ne % 3].dma_start(xgY[d][ds(32 * g, B)], srcY); ne += 1

                for j in range(SC):
                    ps = {}
                    for d in DIRS:
                        ps[d] = (sps.tile([P, NS], F32, tag=f"X_{d}", name=f"X_{d}"),
                                 sps.tile([P, NS], F32, tag=f"Y_{d}", name=f"Y_{d}"))
                        for k in range(KH):
                            for ti in range(2):          # X then Y
                                for g in range(4):
                                    s = 4 * ti + g       # slab index 0..7
                                    nc.tensor.matmul(
                                        ps[d][ti][ds(32 * g, 32)],
                                        lhsT=hT_pad[:, 0 if d == "f" else 1, k],
                                        rhs=whh_sb[d][:, k, ds(NS * s, NS)],
                                        start=(k == 0), stop=(k == KH - 1),
                                        tile_position=(0, 32 * g))
                    hT_ps = sps.tile([P, 2, KH, B], F32, tag="hT_ps", name="hT_ps")
                    for di, d in enumerate(DIRS):
                        X_ps, Y_ps = ps[d]
                        # fold xg (add) straight off PSUM
                        Xf = scp.tile([P, NS], BF16, tag=f"Xf_{d}", name=f"Xf_{d}")
                        nc.vector.tensor_tensor(Xf, X_ps, xgX[d][:, j], ALU.add)
                        Yf = scp.tile([P, NS], BF16, tag=f"Yf_{d}", name=f"Yf_{d}")
                        nc.gpsimd.tensor_tensor(Yf, Y_ps, xgY[d][:, j], ALU.add)
                        # activations: X all sigmoid; Y: tanh (g), sigmoid (o)
                        Xa = scp.tile([P, NS], BF16, tag=f"Xa_{d}", name=f"Xa_{d}")
                        nc.scalar.activation(Xa, Xf, AF.Sigmoid)
                        Ya = scp.tile([P, NS], BF16, tag=f"Ya_{d}", name=f"Ya_{d}")
                        nc.scalar.activation(Ya[0:64], Yf[0:64], AF.Tanh)
                        nc.scalar.activation(Ya[64:128], Yf[64:128], AF.Sigmoid)
                        # cell math on packed [64, NS] (rows 32u+b)
                        t1 = scp.tile([64, NS], BF16, tag=f"t1_{d}", name=f"t1_{d}")
                        nc.vector.tensor_tensor(t1, Xa[0:64], Ya[0:64], ALU.mult)
                        t2 = scp.tile([64, NS], F32, tag=f"t2_{d}", name=f"t2_{d}")
                        nc.gpsimd.tensor_tensor(t2, Xa[64:128], c_pk[d], ALU.mult)
                        nc.vector.tensor_tensor(c_pk[d], t2, t1, ALU.add)
                        th = scp.tile([64, NS], BF16, tag=f"th_{d}", name=f"th_{d}")
                        nc.scalar.activation(th, c_pk[d], AF.Tanh)
                        hpk = scp.tile([64, NS], F32, tag=f"hp_{d}", name=f"hp_{d}")
                        nc.gpsimd.tensor_tensor(hpk, Ya[64:128], th, ALU.mult)
                        # h^T via PE transposes: [4,128] chunks -> [128,4]
                        for u in range(2):
                            for c3 in range(3):
                                nc.tensor.transpose(
                                    hT_ps[:, di, 3 * u + c3],
                                    hpk[ds(32 * u, B), ds(128 * c3, P)],
                                    ident[:B, :B])
                    # update stationary h^T (cols 0..3 of the padded tile)
                    nc.vector.tensor_copy(hT_pad[:, :, :, 0:B], hT_ps)
                    nc.vector.tensor_copy(acc[:, 0, :, :, j], hT_ps[:, 0])
                    nc.scalar.activation(acc[:, 1, :, :, SC - 1 - j], hT_ps[:, 1], AF.Copy)

                # flush acc -> mT_d (one DMA per direction)
                for di, (d, kb) in enumerate((("f", 0), ("b", KH))):
                    t0 = iv * SC if d == "f" else (T - SC) - iv * SC
                    nc.gpsimd.dma_start(
                        mT_d[lname][:, kb:kb + KH, :, ds(t0, SC)],
                        acc[:, di])

    if cfg.get("debug"):
        nc.sync.dma_start(io["dbg_g"], gT_d[:])
        nc.sync.dma_start(io["dbg_m1"], mT_d["l1"][:])
        nc.sync.dma_start(io["dbg_m2"], mT_d["l2"][:])
        nc.sync.dma_start(io["dbg_mo"], mT_d["lo"][:])

    # ================ Phase 3: p1 / p2 ================
    with tc.tile_pool(name="out", bufs=3) as osb, \
         tc.tile_pool(name="out_ps", bufs=2, space="PSUM") as ops:
        for b in range(B):
            p_ps = {nm: ops.tile([1, T], F32, tag=f"{nm}_ps", name=f"{nm}_ps") for nm in ("p1", "p2")}
            for k in range(KH4):
                gt = osb.tile([P, T], BF16, tag="gt")
                nc.sync.dma_start(gt, gT_d[b, k])
                for nm in ("p1", "p2"):
                    nc.tensor.matmul(p_ps[nm], lhsT=pw_sb[f"{nm}_wg"][:, k, None],
                                     rhs=gt, start=(k == 0), stop=False)
            for nm, mt_src in (("p1", mT_d["l2"]), ("p2", mT_d["lo"])):
                for k in range(KH2):
                    mt = osb.tile([P, T], BF16, tag=f"mt_{nm}")
                    nc.sync.dma_start(mt, mt_src[:, k, b])
                    nc.tensor.matmul(p_ps[nm], lhsT=pw_sb[f"{nm}_wm"][:, k, None],
                                     rhs=mt, start=False, stop=(k == KH2 - 1))
            for nm in ("p1", "p2"):
                p_sb = osb.tile([1, T], F32, tag=f"{nm}_sb")
                nc.scalar.activation(p_sb, p_ps[nm], AF.Copy, bias=float(p_bias[nm]))
                nc.sync.dma_start(io[nm][b], p_sb)


# ==================== host-side driver ====================

def _prep_shared(inputs, T):
    """Host-side weight/layout prep shared by all cores."""
    import ml_dtypes
    bf16 = ml_dtypes.bfloat16
    f32 = np.float32
    out = {}
    out["w_cq_"] = np.ascontiguousarray(inputs["w_att_cq"].reshape(KH, P).T).astype(f32)
    out["w_c_"] = np.ascontiguousarray(inputs["w_att_c"].reshape(KH, P).T).astype(f32)
    out["w_q_"] = np.ascontiguousarray(inputs["w_att_q"].reshape(KH, P).T).astype(f32)
    for lname, pre in (("l1", "l1"), ("l2", "l2"), ("lo", "lo")):
        for d in DIRS:
            wih = np.asarray(inputs[f"{pre}{d}_wih"], f32)     # [4H, in]
            whh = np.asarray(inputs[f"{pre}{d}_whh"], f32)     # [4H, H]
            bb = np.asarray(inputs[f"{pre}{d}_b"], f32)        # [4H]
            ind = wih.shape[1]
            out[f"{lname}{d}_wihT"] = np.ascontiguousarray(
                wih.T.reshape(ind // P, P, H4)).astype(bf16)
            out[f"{lname}{d}_whhT"] = np.ascontiguousarray(
                whh.T.reshape(KH, P, H4)).astype(bf16)
            out[f"{lname}{d}_bias"] = bb.reshape(1, H4).copy()
    out["p1_wg_"] = np.ascontiguousarray(inputs["p1_wg"].reshape(KH4, P).T).astype(bf16)
    out["p1_wm_"] = np.ascontiguousarray(inputs["p1_wm"].reshape(KH2, P).T).astype(bf16)
    out["p2_wg_"] = np.ascontiguousarray(inputs["p2_wg"].reshape(KH4, P).T).astype(bf16)
    out["p2_wm_"] = np.ascontiguousarray(inputs["p2_wm"].reshape(KH2, P).T).astype(bf16)
    return out


def _prep_percore(c, q, lo, hi):
    f32 = np.float32
    cs = np.asarray(c[lo:hi], f32)
    qs = np.asarray(q[lo:hi], f32)
    T = cs.shape[1]
    import ml_dtypes
    cT = np.ascontiguousarray(cs.transpose(0, 2, 1).reshape(hi - lo, KH, P, T))
    return {
        "c": np.ascontiguousarray(cs),
        "q": np.ascontiguousarray(qs),
        "cT": cT,
        "cT_bf": cT.astype(ml_dtypes.bfloat16),
        "qT": np.ascontiguousarray(qs.transpose(0, 2, 1).reshape(hi - lo, KH, P, QLEN)),
    }


def declare_io(nc, cfg):
    B, T = cfg["B"], cfg["T"]
    io = {}

    def inp(name, shape, dt=F32):
        io[name] = nc.declare_dram_parameter(name, list(shape), dt, isOutput=False).ap()

    inp("c", (B, T, H))
    inp("q", (B, QLEN, H))
    inp("cT", (B, KH, P, T))
    inp("cT_bf", (B, KH, P, T), BF16)
    inp("qT", (B, KH, P, QLEN))
    inp("w_cq_", (P, KH))
    inp("w_c_", (P, KH))
    inp("w_q_", (P, KH))
    for lname in LAYERS:
        ind = H4 if lname == "l1" else H2
        for d in DIRS:
            inp(f"{lname}{d}_wihT", (ind // P, P, H4), BF16)
            inp(f"{lname}{d}_whhT", (KH, P, H4), BF16)
            inp(f"{lname}{d}_bias", (1, H4))
    inp("p1_wg_", (P, KH4), BF16)
    inp("p1_wm_", (P, KH2), BF16)
    inp("p2_wg_", (P, KH4), BF16)
    inp("p2_wm_", (P, KH2), BF16)
    for nm in ("p1", "p2"):
        io[nm] = nc.declare_dram_parameter(nm, [B, T], F32, isOutput=True).ap()
    return io


def kernel(**inputs):
    from concourse.bass_utils import run_bass_kernel_spmd

    Bloc = B_FULL // N_CORES
    cfg = {
        "B": Bloc, "T": T_FULL, "SC": 8,
        "bias_zero": {f"{l}{d}": not np.any(inputs[f"{l}{d}_b"])
                      for l in LAYERS for d in DIRS},
        "scalars": {
            "b_att": float(inputs["b_att_c"]) + float(inputs["b_att_q"]) + float(inputs["b_att_cq"]),
            "p1_b": float(inputs["p1_bg"]) + float(inputs["p1_bm"]),
            "p2_b": float(inputs["p2_bg"]) + float(inputs["p2_bm"]),
        },
    }

    nc = bacc.Bacc("TRN2", target_bir_lowering=False, debug=False)
    io = declare_io(nc, cfg)
    with tile.TileContext(nc) as tc, ExitStack() as ctx:
        build(ctx, tc, io, cfg)
    nc.compile()

    shared = _prep_shared(inputs, T_FULL)
    in_maps = []
    for core in range(N_CORES):
        m = dict(shared)
        m.update(_prep_percore(inputs["c"], inputs["q"], core * Bloc, (core + 1) * Bloc))
        in_maps.append(m)

    import os as _os
    import time as _time

    def _run():
        try:
            return run_bass_kernel_spmd(nc, in_maps, core_ids=list(range(N_CORES)))
        except Exception:
            if _os.environ.get("BASS_TRACE"):
                _os.environ["BASS_NEVER_TRACE"] = "1"
                return run_bass_kernel_spmd(nc, in_maps, core_ids=list(range(N_CORES)))
            raise

    t0 = _time.time()
    res = _run()
    globals()["LAST_RUN"] = res
    globals()["LAST_EXEC_WALL"] = _time.time() - t0
    t0 = _time.time()
    res2 = _run()
    globals()["WARM_EXEC_WALL"] = _time.time() - t0
    if res2.exec_time_ns is not None:
        globals()["LAST_RUN"] = res2
    res = res2
    p1 = np.concatenate([res.results[i]["p1"] for i in range(N_CORES)], axis=0)
    p2 = np.concatenate([res.results[i]["p2"] for i in range(N_CORES)], axis=0)
    return p1, p2

